# revision 10
# baseline (speedup 1.0000x reference)
"""Trainium2 Bass kernel for CoreferenceResolution.

Math: logits[b,p] = relu(concat(M[b,i], M[b,j], ED[e]) @ W1 + b1) @ W2 + b2
Decomposed as: relu(U[b,i] + V[b,j] + E'[e]) @ W2 + b2 with
  U = M @ W1[:768], V = M @ W1[768:1536], E' = ED @ W1[1536:] + b1.

Per core (8 cores = 2 batches x 4 pair-slices): project the mention table
once on-device (PE, bf16), keep U/V/E' resident in SBUF, then for each
512-pair tile gather rows with SWDGE dma_gather (transposed layout:
hidden on partitions), add + relu, and reduce against W2 on PE.
"""

import sys

sys.path.insert(0, "/opt/trn_rl_repo")

import numpy as np

HIDDEN = 768
HC = 6                       # hidden chunks of 128
B = 2
N_MENT = 2000
MENT_PAD = 2048
M_CHUNKS = 16
N_PAIRS = 40000
ED_PAD = 384                 # 300 ed rows padded
E_CHUNKS = 3
META = 25
W1_ROWS_PAD = 1664           # 1561 -> 13 chunks of 128
W1_CHUNKS = 13
N_CORES = 8
SLICES = 4                   # pair slices per batch
PPC = N_PAIRS // SLICES      # pairs per core = 10000
T = 512                      # pairs per tile
NT = 20                      # tiles per core
PPAD = T * NT                # padded pairs per core = 10240

_COMPILED = None


def _build(phases="abcd"):
    import concourse.mybir as mybir
    import concourse.tile as tile
    from concourse import bacc
    from concourse.bass import ts

    dt = mybir.dt
    nc = bacc.Bacc("TRN2", target_bir_lowering=False, debug=False,
                   num_devices=N_CORES)

    ments_d = nc.dram_tensor("ments", [MENT_PAD, HIDDEN], dt.float32,
                             kind="ExternalInput").ap()
    w1_d = nc.dram_tensor("w1p", [W1_ROWS_PAD, HIDDEN], dt.float32,
                          kind="ExternalInput").ap()
    w2_d = nc.dram_tensor("w2", [HIDDEN], dt.float32,
                          kind="ExternalInput").ap()
    b1_d = nc.dram_tensor("b1", [HIDDEN], dt.float32,
                          kind="ExternalInput").ap()
    b2_d = nc.dram_tensor("b2", [1], dt.float32, kind="ExternalInput").ap()
    edt_d = nc.dram_tensor("edt", [32, ED_PAD], dt.float32,
                           kind="ExternalInput").ap()
    idxa_d = nc.dram_tensor("idxa", [128, PPAD // 16], dt.int16,
                            kind="ExternalInput").ap()
    idxb_d = nc.dram_tensor("idxb", [128, PPAD // 16], dt.int16,
                            kind="ExternalInput").ap()
    idxe_d = nc.dram_tensor("idxe", [128, PPAD // 16], dt.int16,
                            kind="ExternalInput").ap()
    iota_d = nc.dram_tensor("iota", [128, MENT_PAD // 16], dt.int16,
                            kind="ExternalInput").ap()
    out_d = nc.dram_tensor("out", [PPAD], dt.float32,
                           kind="ExternalOutput").ap()

    ROW_B = HIDDEN * 2       # bf16 table row bytes (1536)

    with tile.TileContext(nc) as tc:
        with (
            tc.tile_pool(name="const", bufs=1) as cpool,
            tc.tile_pool(name="tables", bufs=1) as tpool,
        ):
            w1_sb = cpool.tile([128, W1_CHUNKS * HIDDEN], dt.bfloat16)
            w2b = cpool.tile([128, HC], dt.bfloat16)
            b1b = cpool.tile([128, HIDDEN], dt.float32)
            b2_sb = cpool.tile([1, 1], dt.float32)
            edt_sb = cpool.tile([32, ED_PAD], dt.bfloat16)
            idxa_sb = cpool.tile([128, PPAD // 16], dt.int16)
            idxb_sb = cpool.tile([128, PPAD // 16], dt.int16)
            idxe_sb = cpool.tile([128, PPAD // 16], dt.int16)
            iota_sb = cpool.tile([128, MENT_PAD // 16], dt.int16)

            u_sb = tpool.tile([128, M_CHUNKS * HIDDEN], dt.bfloat16)
            v_sb = tpool.tile([128, M_CHUNKS * HIDDEN], dt.bfloat16)
            e_sb = tpool.tile([128, E_CHUNKS * HIDDEN], dt.bfloat16)
            # mentions^T in 4 groups of 512 ments (gather num_idxs cap)
            mentT = [tpool.tile([128, HC, 512], dt.bfloat16, tag=f"mT{g}",
                                name=f"mentT{g}")
                     for g in range(4)]

            # ---- Phase A: load constants / stage + cast inputs ----
            nc.sync.dma_start(b2_sb[:], b2_d[:])
            nc.sync.dma_start(idxa_sb[:], idxa_d[:])
            nc.sync.dma_start(idxb_sb[:], idxb_d[:])
            nc.sync.dma_start(idxe_sb[:], idxe_d[:])
            nc.sync.dma_start(iota_sb[:], iota_d[:])

            with (
                tc.tile_pool(name="stage", bufs=4) as spool,
                tc.tile_pool(name="small", bufs=1) as smpool,
                tc.tile_pool(name="mstage", bufs=1) as mpool,
                tc.tile_pool(name="psA", bufs=4, space="PSUM") as psA,
            ):
                # W1 -> bf16 (13 chunks of 128 input-dims)
                for c in range(W1_CHUNKS):
                    st = spool.tile([128, HIDDEN], dt.float32, tag="st")
                    nc.sync.dma_start(st[:], w1_d[ts(c, 128), :])
                    nc.scalar.copy(w1_sb[:, ts(c, HIDDEN)], st[:])

                # mentions -> bf16 table layout [p=m%128, (m//128)*768]
                ment_stage = mpool.tile([128, M_CHUNKS * HIDDEN], dt.bfloat16)
                for r in range(M_CHUNKS):
                    st = spool.tile([128, HIDDEN], dt.float32, tag="st")
                    nc.sync.dma_start(st[:], ments_d[ts(r, 128), :])
                    nc.vector.tensor_copy(ment_stage[:, ts(r, HIDDEN)], st[:])

                # mentions^T via SBUF-source gather with iota indices
                for g in range(4):
                    nc.gpsimd.dma_gather(
                        mentT[g][:], ment_stage[:],
                        iota_sb[:, g * 32:(g + 1) * 32],
                        512, 512, HIDDEN, transpose=True,
                        sbuf_tokens_per_rank=128,
                        sbuf_free_dim_per_rank=ROW_B,
                    )

                # W2 -> [p, c] = W2[c*128+p], bf16
                w2st = smpool.tile([128, HC], dt.float32)
                nc.sync.dma_start(w2st[:], w2_d.rearrange("(c p) -> p c", p=128))
                nc.vector.tensor_copy(w2b[:], w2st[:])

                # b1 broadcast to all partitions (fp32)
                b1st = smpool.tile([1, HIDDEN], dt.float32)
                nc.sync.dma_start(b1st[:], b1_d[None, :])
                nc.gpsimd.partition_broadcast(b1b[:], b1st[:])

                # ed_table^T -> bf16
                edst = smpool.tile([32, ED_PAD], dt.float32)
                nc.sync.dma_start(edst[:], edt_d[:])
                nc.vector.tensor_copy(edt_sb[:], edst[:])

                # ---- Phase C (tiny, first): E' = ed^T.T @ W1c + b1 ----
                w1c_off = 12 * HIDDEN
                for m in range(E_CHUNKS if "c" in phases else 0):
                    p5 = psA.tile([128, 512], dt.float32, tag="p5")
                    p2 = psA.tile([128, 256], dt.float32, tag="p2")
                    lhs = edt_sb[:META, ts(m, 128)]
                    nc.tensor.matmul(p5[:], lhs, w1_sb[:META, w1c_off:w1c_off + 512],
                                     start=True, stop=True)
                    nc.tensor.matmul(p2[:], lhs, w1_sb[:META, w1c_off + 512:w1c_off + HIDDEN],
                                     start=True, stop=True)
                    nc.vector.tensor_add(e_sb[:, m * HIDDEN:m * HIDDEN + 512],
                                         p5[:], b1b[:, :512])
                    nc.vector.tensor_add(e_sb[:, m * HIDDEN + 512:(m + 1) * HIDDEN],
                                         p2[:], b1b[:, 512:])

                # ---- Phase B: U/V projections ----
                for r in range(M_CHUNKS if "b" in phases else 0):
                    u5 = psA.tile([128, 512], dt.float32, tag="p5")
                    u2 = psA.tile([128, 256], dt.float32, tag="p2")
                    v5 = psA.tile([128, 512], dt.float32, tag="p5")
                    v2 = psA.tile([128, 256], dt.float32, tag="p2")
                    for k in range(HC):
                        lhs = mentT[r // 4][:, k, ts(r % 4, 128)]
                        st0, sp1 = (k == 0), (k == HC - 1)
                        ua = k * HIDDEN
                        va = (HC + k) * HIDDEN
                        nc.tensor.matmul(u5[:], lhs, w1_sb[:, ua:ua + 512],
                                         start=st0, stop=sp1)
                        nc.tensor.matmul(u2[:], lhs, w1_sb[:, ua + 512:ua + HIDDEN],
                                         start=st0, stop=sp1)
                        nc.tensor.matmul(v5[:], lhs, w1_sb[:, va:va + 512],
                                         start=st0, stop=sp1)
                        nc.tensor.matmul(v2[:], lhs, w1_sb[:, va + 512:va + HIDDEN],
                                         start=st0, stop=sp1)
                    ro = r * HIDDEN
                    nc.vector.tensor_copy(u_sb[:, ro:ro + 512], u5[:])
                    nc.vector.tensor_copy(u_sb[:, ro + 512:ro + HIDDEN], u2[:])
                    nc.scalar.copy(v_sb[:, ro:ro + 512], v5[:])
                    nc.scalar.copy(v_sb[:, ro + 512:ro + HIDDEN], v2[:])

            # ---- Phase D: gather + add + relu + dot per 512-pair tile ----
            with (
                tc.tile_pool(name="g", bufs=2) as gpool,
                tc.tile_pool(name="o", bufs=2) as opool,
                tc.tile_pool(name="psD", bufs=4, space="PSUM") as psD,
            ):
                relu = mybir.ActivationFunctionType.Relu
                ident = mybir.ActivationFunctionType.Identity
                if "d" not in phases:
                    for t in range(NT):
                        lt = opool.tile([1, T], dt.float32, tag="lt")
                        nc.vector.memset(lt[:], 0.0)
                        nc.sync.dma_start(out_d[ts(t, T)], lt[:])
                for t in range(NT if "d" in phases else 0):
                    isl = (slice(None), slice(t * (T // 16), (t + 1) * (T // 16)))
                    gu = gpool.tile([128, HC, T], dt.bfloat16, tag="gu")
                    gv = gpool.tile([128, HC, T], dt.bfloat16, tag="gv")
                    ge = gpool.tile([128, HC, T], dt.bfloat16, tag="ge")
                    nc.gpsimd.dma_gather(
                        gu[:], u_sb[:], idxa_sb[isl], T, T, HIDDEN,
                        transpose=True, sbuf_tokens_per_rank=128,
                        sbuf_free_dim_per_rank=ROW_B)
                    nc.gpsimd.dma_gather(
                        gv[:], v_sb[:], idxb_sb[isl], T, T, HIDDEN,
                        transpose=True, sbuf_tokens_per_rank=128,
                        sbuf_free_dim_per_rank=ROW_B)
                    nc.gpsimd.dma_gather(
                        ge[:], e_sb[:], idxe_sb[isl], T, T, HIDDEN,
                        transpose=True, sbuf_tokens_per_rank=128,
                        sbuf_free_dim_per_rank=ROW_B)
                    nc.vector.tensor_add(gu[:], gu[:], gv[:])
                    nc.vector.tensor_add(gu[:], gu[:], ge[:])
                    nc.scalar.activation(gu[:], gu[:], relu)
                    pl = psD.tile([1, T], dt.float32, tag="pl")
                    for c in range(HC):
                        nc.tensor.matmul(pl[:], w2b[:, c:c + 1], gu[:, c, :],
                                         start=(c == 0), stop=(c == HC - 1))
                    lt = opool.tile([1, T], dt.float32, tag="lt")
                    nc.scalar.activation(lt[:], pl[:], ident, bias=b2_sb[:1, :1])
                    nc.sync.dma_start(out_d[ts(t, T)], lt[:])

    nc.compile()
    return nc


def _get_compiled():
    global _COMPILED
    if _COMPILED is None:
        _COMPILED = _build()
    return _COMPILED


def _wrap16(x):
    """idx i -> [i % 16, i // 16] int16 layout for dma_gather."""
    x = np.asarray(x, dtype=np.int16)
    w = np.ascontiguousarray(x.reshape(-1, 16).T)
    return np.tile(w, (8, 1))


def make_in_maps(mention_reprs, coref_mention_pairs, coref_eds, ed_table,
                 W1, b1, W2, b2):
    mention_reprs = np.asarray(mention_reprs, dtype=np.float32)
    pairs = np.asarray(coref_mention_pairs)
    eds = np.asarray(coref_eds)
    W1 = np.asarray(W1, dtype=np.float32)
    W2 = np.asarray(W2, dtype=np.float32)
    b1 = np.asarray(b1, dtype=np.float32)
    b2 = np.asarray(b2, dtype=np.float32)
    ed_table = np.asarray(ed_table, dtype=np.float32)

    w1p = np.zeros((W1_ROWS_PAD, HIDDEN), np.float32)
    w1p[:W1.shape[0]] = W1
    edt = np.zeros((32, ED_PAD), np.float32)
    edt[:META, :ed_table.shape[0]] = ed_table.T
    iota = _wrap16(np.arange(MENT_PAD))

    shared = {
        "w1p": w1p,
        "w2": W2.reshape(HIDDEN),
        "b1": b1.reshape(HIDDEN),
        "b2": b2.reshape(1),
        "edt": edt,
        "iota": iota,
    }

    in_maps = []
    for core in range(N_CORES):
        b = core // SLICES
        q = core % SLICES
        ments = np.zeros((MENT_PAD, HIDDEN), np.float32)
        ments[:N_MENT] = mention_reprs[b]
        sl = slice(q * PPC, (q + 1) * PPC)
        ia = np.zeros(PPAD, np.int64)
        ib = np.zeros(PPAD, np.int64)
        ie = np.zeros(PPAD, np.int64)
        ia[:PPC] = pairs[b, sl, 0]
        ib[:PPC] = pairs[b, sl, 1]
        ie[:PPC] = eds[b, sl]
        in_maps.append({
            "ments": ments,
            "idxa": _wrap16(ia),
            "idxb": _wrap16(ib),
            "idxe": _wrap16(ie),
            **shared,
        })
    return in_maps


def unshard(results):
    out = np.zeros((B, N_PAIRS), np.float32)
    for core in range(N_CORES):
        b = core // SLICES
        q = core % SLICES
        out[b, q * PPC:(q + 1) * PPC] = results[core]["out"][:PPC]
    return out


def kernel(**inputs):
    from concourse.bass_utils import run_bass_kernel_spmd

    nc = _get_compiled()
    in_maps = make_in_maps(**inputs)
    res = run_bass_kernel_spmd(nc, in_maps, list(range(N_CORES)))
    return unshard(res.results)
